# revision 3
# baseline (speedup 1.0000x reference)
"""DotInteraction Trainium2 kernel.

features [16384, 27, 128] f32 -> strict-lower-triangle pairwise dots [16384, 351].

Pure data parallel over batch: 2048 samples per core on 8 cores. Per core,
samples are processed in groups of 4 (108 feature rows):
  1. One contiguous DMA loads the group's X rows -> SBUF a [108, 128].
  2. One PE transpose (vs a constant identity) -> PSUM pt [128, 108] = X^T,
     sample j at cols 27j..27j+27.
  3. One ScalarE copy PSUM -> SBUF (xt).
  4. 4 col-tiled PE matmuls (concurrent on the 128x128 array) compute
     G_j = X_j @ X_j^T. Matmul j streams rhs = xt[:, 27j:27j+27] and loads a
     32-wide lhsT window at col LHS_OFF[j], so each 32-partition PSUM strip
     is fully written (extra rows are neighboring samples' products,
     discarded). Sample j's G rows land at partition ROW_OFF[j].
  5. One VectorE copy PSUM -> SBUF, one contiguous DMA out [128, 27].
Host slices the 27x27 blocks out of the dump and gathers tril indices.
"""
import numpy as np

B, F, D = 16384, 27, 128
NCORES = 8
BL = B // NCORES            # samples per core
GRP = 4                     # samples per group
NG = BL // GRP              # groups per core
LHS_OFF = [0, 27, 54, 76]   # 32-wide lhsT col windows within the 108-col xt
ROW_OFF = [0, 32, 64, 101]  # sample j's G rows at this partition offset

_CACHE = {}


def _build():
    import concourse.tile as tile
    from concourse import bacc, mybir

    f32 = mybir.dt.float32
    nc = bacc.Bacc("TRN2", target_bir_lowering=False, debug=False)
    feat = nc.dram_tensor("features", [BL * F, D], f32, kind="ExternalInput")
    ident_d = nc.dram_tensor("ident", [GRP * F, GRP * F], f32,
                             kind="ExternalInput")
    out_d = nc.dram_tensor("out", [NG, 128, F], f32, kind="ExternalOutput")

    with tile.TileContext(nc) as tc:
        with (
            tc.tile_pool(name="const", bufs=1) as const_pool,
            tc.tile_pool(name="a", bufs=4) as a_pool,
            tc.tile_pool(name="xt", bufs=4) as xt_pool,
            tc.tile_pool(name="gs", bufs=4) as gs_pool,
            tc.tile_pool(name="pt", bufs=2, space="PSUM") as pt_pool,
            tc.tile_pool(name="pg", bufs=2, space="PSUM") as pg_pool,
        ):
            ident = const_pool.tile([GRP * F, GRP * F], f32)
            nc.sync.dma_start(ident[:], ident_d[:])

            for g in range(NG):
                a = a_pool.tile([GRP * F, D], f32)
                nc.sync.dma_start(a[:], feat[108 * g:108 * (g + 1), :])

                pt = pt_pool.tile([128, GRP * F], f32)
                nc.tensor.transpose(pt[:], a[:], ident[:])

                xt = xt_pool.tile([128, GRP * F], f32)
                nc.scalar.copy(xt[:], pt[:])

                pg = pg_pool.tile([128, F], f32)
                for j in range(GRP):
                    nc.tensor.matmul(
                        pg[32 * j:32 * (j + 1), :],
                        xt[:, LHS_OFF[j]:LHS_OFF[j] + 32],
                        xt[:, F * j:F * (j + 1)],
                        tile_position=(0, 32 * j),
                    )

                gs = gs_pool.tile([128, F], f32)
                nc.vector.tensor_copy(gs[:], pg[:])

                nc.sync.dma_start(out_d[g], gs[:])

    nc.compile()
    return nc


def _run_spmd(nc, in_maps):
    """Like bass2jax.run_bass_via_pjrt multi-core, but builds the global
    sharded arrays from per-device shards (device_put per core) instead of
    one host concat — a single large host->device transfer can fail on the
    axon relay; per-core transfers are fine."""
    import jax
    from jax.experimental.shard_map import shard_map
    from jax.sharding import Mesh, NamedSharding, PartitionSpec
    from concourse import bass2jax, mybir

    bass2jax.install_neuronx_cc_hook()
    partition_name = (nc.partition_id_tensor.name
                      if nc.partition_id_tensor else None)
    in_names, out_names, out_avals = [], [], []
    for alloc in nc.m.functions[0].allocations:
        if not isinstance(alloc, mybir.MemoryLocationSet):
            continue
        name = alloc.memorylocations[0].name
        if alloc.kind == "ExternalInput":
            if name != partition_name:
                in_names.append(name)
        elif alloc.kind == "ExternalOutput":
            out_names.append(name)
            out_avals.append(jax.core.ShapedArray(
                tuple(alloc.tensor_shape), mybir.dt.np(alloc.dtype)))
    n_params = len(in_names)
    n_outs = len(out_names)
    all_in_names = list(in_names) + list(out_names)
    if partition_name is not None:
        all_in_names.append(partition_name)

    def _body(*args):
        operands = list(args)
        if partition_name is not None:
            operands.append(bass2jax.partition_id_tensor())
        outs = bass2jax._bass_exec_p.bind(
            *operands,
            out_avals=tuple(out_avals),
            in_names=tuple(all_in_names),
            out_names=tuple(out_names),
            lowering_input_output_aliases=(),
            sim_require_finite=True,
            sim_require_nnan=True,
            nc=nc,
        )
        return tuple(outs)

    devices = jax.devices()[:NCORES]
    mesh = Mesh(np.asarray(devices), ("core",))
    sharding = NamedSharding(mesh, PartitionSpec("core"))
    donate = tuple(range(n_params, n_params + n_outs))
    sharded = jax.jit(
        shard_map(_body, mesh=mesh,
                  in_specs=(PartitionSpec("core"),) * (n_params + n_outs),
                  out_specs=(PartitionSpec("core"),) * n_outs,
                  check_rep=False),
        donate_argnums=donate, keep_unused=True)

    def _global(per_core):
        shards = [jax.device_put(per_core[c], devices[c])
                  for c in range(NCORES)]
        gshape = (NCORES * per_core[0].shape[0], *per_core[0].shape[1:])
        return jax.make_array_from_single_device_arrays(
            gshape, sharding, shards)

    gins = [_global([np.asarray(m[name]) for m in in_maps])
            for name in in_names]
    gzeros = [_global([np.zeros(av.shape, av.dtype)] * NCORES)
              for av in out_avals]
    out_arrs = sharded(*gins, *gzeros)

    fetched = [np.asarray(a).reshape(NCORES, *out_avals[i].shape)
               for i, a in enumerate(out_arrs)]
    return [{name: fetched[i][c] for i, name in enumerate(out_names)}
            for c in range(NCORES)]


def kernel(features: np.ndarray) -> np.ndarray:
    features = np.ascontiguousarray(np.asarray(features, dtype=np.float32))
    assert features.shape == (B, F, D), features.shape

    if "nc" not in _CACHE:
        _CACHE["nc"] = _build()
    nc = _CACHE["nc"]

    ident = np.eye(GRP * F, dtype=np.float32)
    flat = features.reshape(B * F, D)
    in_maps = [{"features": flat[c * BL * F:(c + 1) * BL * F],
                "ident": ident} for c in range(NCORES)]

    results = _run_spmd(nc, in_maps)

    # [NCORES][NG, 128, F] -> [B, F, F]
    gfull = np.empty((B, F, F), dtype=np.float32)
    for c in range(NCORES):
        dump = results[c]["out"]                # [NG, 128, F]
        blocks = np.empty((NG, GRP, F, F), dtype=np.float32)
        for j in range(GRP):
            blocks[:, j] = dump[:, ROW_OFF[j]:ROW_OFF[j] + F, :]
        gfull[c * BL:(c + 1) * BL] = blocks.reshape(BL, F, F)

    rows, cols = np.tril_indices(F, k=-1)
    return np.ascontiguousarray(gfull[:, rows, cols])
